# revision 7
# baseline (speedup 1.0000x reference)
"""Trainium2 Bass kernel for nn_Attention_75505525063766.

Sharding: 8 cores; core c handles batch b = c//2 and query-row half
ihalf = c%2 (512 rows) with all 8 heads.  Masks/weights replicated.
Global statistics (sigma = mean(dist), wmean = mean(w)) are combined
across cores with two scalar AllReduces.

On-chip layout is "transposed" throughout — features / key-index j on
partitions, the core's 512 query rows i on the free axis — so no
on-device transposes are needed anywhere.
"""

import os
import numpy as np

import concourse.bass as bass
import concourse.bacc as bacc
import concourse.mybir as mybir
import concourse.tile as tile
from concourse.bass_utils import run_bass_kernel_spmd

B, N, D = 4, 1024, 64
H, DH = 8, 64
INNER = 512
NUM_F, GH, GW = 4, 16, 16
EPS_LN = 1e-5
P = 128
IW = 512          # query rows per core
NJT = N // P      # 8 j-tiles
NCORES = 8
BN2 = float(B * N * N)
CLAMP = 1e-30

F32 = mybir.dt.float32
F32R = mybir.dt.float32r
BF16 = mybir.dt.bfloat16
AF = mybir.ActivationFunctionType
ALU = mybir.AluOpType
AX = mybir.AxisListType

DT_BIG = F32R      # qkv, dots, LN stats, out proj
E_DT = BF16        # exp(masked) and v_aug dtype for the attn@v matmul

_CACHE = {}


def _spa_full():
    """Input-independent spatial mask (fp32 replica of the reference)."""
    gx, gy, gz = np.meshgrid(np.arange(GH), np.arange(GW), np.arange(NUM_F),
                             indexing="ij")
    grid = np.stack([gx, gy, gz], -1).reshape(N, 3).astype(np.float32)
    diff = grid[:, None, :] - grid[None, :, :]
    d2 = (diff * diff).sum(-1)
    dist = np.sqrt(d2)
    sigma = dist.mean(dtype=np.float32)
    w = np.exp(-d2 / (2.0 * sigma * sigma)).astype(np.float32)
    return np.where(w < w.mean(dtype=np.float32), 0.0, w).astype(np.float32)


def build_nc():
    nc = bacc.Bacc("TRN2", target_bir_lowering=False, debug=False,
                   num_devices=NCORES)

    def _r(x):
        return x.bitcast(DT_BIG) if DT_BIG != F32 else x

    xT_e = nc.dram_tensor("xT", [D, N], F32, kind="ExternalInput")
    xTmy_e = nc.dram_tensor("xTmy", [D, IW], F32, kind="ExternalInput")
    W_e = nc.dram_tensor("Wqkv", [D, 3 * INNER], F32, kind="ExternalInput")
    bqk_e = nc.dram_tensor("bqk", [P, 8], F32, kind="ExternalInput")
    bv_e = nc.dram_tensor("bv", [1, INNER], F32, kind="ExternalInput")
    spa_e = nc.dram_tensor("spa", [N, IW], F32, kind="ExternalInput")
    lng_e = nc.dram_tensor("lng", [64, H], F32, kind="ExternalInput")
    lnb_e = nc.dram_tensor("lnb", [64, H], F32, kind="ExternalInput")
    Wout_e = nc.dram_tensor("Wout", [64, H * D], F32, kind="ExternalInput")
    bout_e = nc.dram_tensor("bout", [D, 1], F32, kind="ExternalInput")

    out_e = nc.dram_tensor("out", [D, IW], F32, kind="ExternalOutput")
    dbg_e = nc.dram_tensor("dbg", [1, 16], F32, kind="ExternalOutput")

    RG = [list(range(NCORES))]

    with tile.TileContext(nc) as tc:
        with tc.tile_pool(name="const", bufs=1) as cp, \
             tc.tile_pool(name="blk", bufs=1) as bp, \
             tc.tile_pool(name="scr", bufs=3) as sp, \
             tc.tile_pool(name="row", bufs=1) as rp, \
             tc.tile_pool(name="psA", bufs=2, space="PSUM") as psA, \
             tc.tile_pool(name="psB", bufs=2, space="PSUM") as psB, \
             tc.tile_pool(name="psR", bufs=2, space="PSUM") as psR, \
             tc.tile_pool(name="dram", bufs=1, space="DRAM") as dp:

            # ---------------- inputs to SBUF ----------------
            xT = cp.tile([D, N], F32, tag="xT")
            nc.sync.dma_start(out=xT[:], in_=xT_e[:])
            xTmy = cp.tile([D, IW], F32, tag="xTmy")
            nc.sync.dma_start(out=xTmy[:], in_=xTmy_e[:])
            Wsb = cp.tile([D, 3 * INNER], F32, tag="W")
            nc.sync.dma_start(out=Wsb[:], in_=W_e[:])
            bqk = cp.tile([P, 8], F32, tag="bqk")
            nc.sync.dma_start(out=bqk[:], in_=bqk_e[:])
            bv = cp.tile([1, INNER], F32, tag="bv")
            nc.sync.dma_start(out=bv[:], in_=bv_e[:])
            lng = cp.tile([64, H], F32, tag="lng")
            nc.sync.dma_start(out=lng[:], in_=lng_e[:])
            lnb = cp.tile([64, H], F32, tag="lnb")
            nc.sync.dma_start(out=lnb[:], in_=lnb_e[:])
            Wout = cp.tile([64, H * D], F32, tag="Wout")
            nc.sync.dma_start(out=Wout[:], in_=Wout_e[:])
            bout = cp.tile([D, 1], F32, tag="bout")
            nc.sync.dma_start(out=bout[:], in_=bout_e[:])

            Wr = cp.tile([D, 3 * INNER], F32R, tag="Wr")
            nc.vector.tensor_copy(Wr[:], Wsb[:])
            xTr = cp.tile([D, N], F32R, tag="xTr")
            nc.vector.tensor_copy(xTr[:], xT[:])
            xTmyr = cp.tile([D, IW], F32R, tag="xTmyr")
            nc.vector.tensor_copy(xTmyr[:], xTmy[:])

            ones_col = cp.tile([P, 1], F32, tag="ones_col")
            nc.vector.memset(ones_col[:], 1.0)
            ones_row = cp.tile([1, P], F32, tag="ones_row")
            nc.vector.memset(ones_row[:], 1.0)

            # xTm2 = -2 * xT  (lhsT of the d2 dot matmul)
            xTm2 = cp.tile([D, N], F32, tag="xTm2")
            nc.vector.tensor_scalar_mul(xTm2[:], xT[:], -2.0)

            # ---------------- squared norms ----------------
            # sq_lhs: row0 = |x_j|^2 (1,N), row1 = ones
            # sq_rhs: row0 = ones,          row1 = |x_i|^2 (my rows)
            sq_lhs = rp.tile([2, N], F32, tag="sq_lhs")
            sq_rhs = rp.tile([2, IW], F32, tag="sq_rhs")
            ones_rowN = rp.tile([1, N], F32, tag="ones_rowN")
            nc.vector.memset(ones_rowN[:], 1.0)
            nc.sync.dma_start(out=sq_lhs[1:2, :], in_=ones_rowN[:])
            nc.vector.memset(sq_rhs[0:1, :], 1.0)
            xsq = sp.tile([D, N], F32, tag="s4")
            nc.vector.tensor_mul(xsq[:], xT[:], xT[:])
            for hh in range(2):
                ps = psR.tile([1, IW], F32, tag="row")
                nc.tensor.matmul(ps[:], ones_col[0:D, :],
                                 xsq[:, hh * IW:(hh + 1) * IW],
                                 start=True, stop=True)
                nc.scalar.copy(sq_lhs[0:1, hh * IW:(hh + 1) * IW], ps[:])
            xsqmy = sp.tile([D, IW], F32, tag="s2")
            nc.vector.tensor_mul(xsqmy[:], xTmy[:], xTmy[:])
            sqi_ps = psR.tile([1, IW], F32, tag="row")
            nc.tensor.matmul(sqi_ps[:], ones_col[0:D, :], xsqmy[:],
                             start=True, stop=True)
            sqi_sb = rp.tile([1, IW], F32, tag="sqi")
            nc.scalar.copy(sqi_sb[:], sqi_ps[:])
            nc.sync.dma_start(out=sq_rhs[1:2, :], in_=sqi_sb[:])

            # ---------------- d2 blocks + dist partial sums ----------------
            d2r = []
            dist_acc = rp.tile([P, NJT], F32, tag="dist_acc")
            for jt in range(NJT):
                ps = psA.tile([P, 2 * IW], F32, tag="mm")
                nc.tensor.matmul(ps[:, 0:IW], xTm2[:, jt * P:(jt + 1) * P],
                                 xTmy[:], start=True, stop=False)
                nc.tensor.matmul(ps[:, 0:IW], sq_lhs[:, jt * P:(jt + 1) * P],
                                 sq_rhs[:], start=False, stop=True)
                t = bp.tile([P, IW], F32, tag=f"d2r{jt}")
                # clamp to CLAMP (>0) so Ln stays finite; acts as the relu
                nc.vector.tensor_scalar_max(t[:], ps[:, 0:IW], CLAMP)
                d2r.append(t)
                # dist = exp(0.5*ln(d2)); accumulate per-partition row sums
                lg = sp.tile([P, IW], F32, tag="s2")
                nc.scalar.activation(lg[:], t[:], AF.Ln)
                dst = sp.tile([P, IW], F32, tag="s2")
                nc.scalar.activation(dst[:], lg[:], AF.Exp, scale=0.5,
                                     accum_out=dist_acc[:, jt:jt + 1])

            dist_red = rp.tile([P, 1], F32, tag="dist_red")
            nc.vector.tensor_reduce(dist_red[:], dist_acc[:], axis=AX.X,
                                    op=ALU.add)
            tot_ps = psR.tile([1, 1], F32, tag="row")
            nc.tensor.matmul(tot_ps[:], ones_col[:], dist_red[:],
                             start=True, stop=True)
            ar1_sb = rp.tile([1, 16], F32, tag="ar1")
            nc.vector.memset(ar1_sb[:], 0.0)
            nc.scalar.copy(ar1_sb[:, 0:1], tot_ps[:])
            ar1_in = dp.tile([1, 16], F32, tag="ar1_in")
            ar1_out = dp.tile([1, 16], F32, tag="ar1_out", addr_space="Shared")
            nc.sync.dma_start(out=ar1_in[:], in_=ar1_sb[:])
            nc.gpsimd.collective_compute(
                "AllReduce", ALU.add, replica_groups=RG,
                ins=[ar1_in[:].opt()], outs=[ar1_out[:].opt()])

            # ---------------- qkv + elu (overlaps the collective) --------
            # elu(x)+1 = min(exp(x),1) + relu(x)
            qkT = []
            for fc in range(8):
                isq = fc < 4
                nrow = IW if isq else N
                rhs = xTmyr if isq else xTr
                ps = psA.tile([P, 2 * IW], F32, tag="mm")
                for hh in range(nrow // IW):
                    nc.tensor.matmul(ps[:, hh * IW:(hh + 1) * IW],
                                     Wr[:, fc * P:(fc + 1) * P],
                                     rhs[:, hh * IW:(hh + 1) * IW],
                                     start=True, stop=True)
                ex = sp.tile([P, N], F32, tag="s4")
                nc.scalar.activation(ex[:, 0:nrow], ps[:, 0:nrow], AF.Exp,
                                     bias=bqk[:, fc:fc + 1])
                rl = sp.tile([P, N], F32, tag="s4")
                nc.scalar.activation(rl[:, 0:nrow], ps[:, 0:nrow], AF.Relu,
                                     bias=bqk[:, fc:fc + 1])
                dst = bp.tile([P, nrow], F32R, tag=f"qk{fc}")
                nc.vector.scalar_tensor_tensor(dst[:], ex[:, 0:nrow], 1.0,
                                               rl[:, 0:nrow],
                                               op0=ALU.min, op1=ALU.add)
                qkT.append(dst)
            qeT, keT = qkT[:4], qkT[4:]

            # v in row layout with bias, 65-column augmented (ones) bf16
            v_aug = []
            for rt in range(NJT):
                ps = psB.tile([P, IW], F32, tag="out")
                nc.tensor.matmul(ps[:], xTr[:, rt * P:(rt + 1) * P],
                                 Wr[:, 2 * INNER:3 * INNER],
                                 start=True, stop=False)
                nc.tensor.matmul(ps[:], ones_row[:], bv[:],
                                 start=False, stop=True)
                va = bp.tile([P, 8 * 65], E_DT, tag=f"vaug{rt}")
                nc.vector.memset(va[:], 1.0)
                nc.scalar.copy(
                    va[:].rearrange("p (h c) -> p h c", c=65)[:, :, 0:64],
                    ps[:].rearrange("p (h c) -> p h c", c=64))
                v_aug.append(va)

            # ---------------- sigma -> exp scale column ----------------
            ar1_back = rp.tile([1, 16], F32, tag="ar1b")
            nc.sync.dma_start(out=ar1_back[:], in_=ar1_out[:])
            inv_s1 = rp.tile([1, 1], F32, tag="invs1")
            nc.vector.reciprocal(inv_s1[:], ar1_back[:, 0:1])
            sc0 = rp.tile([1, 1], F32, tag="sc0")
            nc.scalar.activation(sc0[:], inv_s1[:], AF.Square, scale=float(BN2))
            scale_val = rp.tile([1, 1], F32, tag="scv")
            nc.scalar.mul(scale_val[:], sc0[:], -0.5)   # -1/(2 sigma^2)
            col_ps = psR.tile([P, 1], F32, tag="row")
            nc.tensor.matmul(col_ps[:], ones_row[:], scale_val[:],
                             start=True, stop=True)
            scale_col = rp.tile([P, 1], F32, tag="scale_col")
            nc.scalar.copy(scale_col[:], col_ps[:])

            # ---------------- w = exp(-d2/(2 s^2)); partial sum ----------
            w_t = []
            w_acc = rp.tile([P, NJT], F32, tag="w_acc")
            for jt in range(NJT):
                t = bp.tile([P, IW], F32, tag=f"w{jt}")
                nc.scalar.activation(t[:], d2r[jt][:], AF.Exp,
                                     scale=scale_col[:],
                                     accum_out=w_acc[:, jt:jt + 1])
                w_t.append(t)
            w_red = rp.tile([P, 1], F32, tag="w_red")
            nc.vector.tensor_reduce(w_red[:], w_acc[:], axis=AX.X, op=ALU.add)
            tot2_ps = psR.tile([1, 1], F32, tag="row")
            nc.tensor.matmul(tot2_ps[:], ones_col[:], w_red[:],
                             start=True, stop=True)
            ar2_sb = rp.tile([1, 16], F32, tag="ar2")
            nc.vector.memset(ar2_sb[:], 0.0)
            nc.scalar.copy(ar2_sb[:, 0:1], tot2_ps[:])
            ar2_in = dp.tile([1, 16], F32, tag="ar2_in")
            ar2_out = dp.tile([1, 16], F32, tag="ar2_out", addr_space="Shared")
            nc.sync.dma_start(out=ar2_in[:], in_=ar2_sb[:])
            nc.gpsimd.collective_compute(
                "AllReduce", ALU.add, replica_groups=RG,
                ins=[ar2_in[:].opt()], outs=[ar2_out[:].opt()])
            ar2_back = rp.tile([1, 16], F32, tag="ar2b")
            nc.sync.dma_start(out=ar2_back[:], in_=ar2_out[:])
            wm_val = rp.tile([1, 1], F32, tag="wm_val")
            nc.scalar.mul(wm_val[:], ar2_back[:, 0:1], 1.0 / BN2)
            col2_ps = psR.tile([P, 1], F32, tag="row")
            nc.tensor.matmul(col2_ps[:], ones_row[:], wm_val[:],
                             start=True, stop=True)
            wm_col = rp.tile([P, 1], F32, tag="wm_col")
            nc.scalar.copy(wm_col[:], col2_ps[:])

            # ---------------- mask m = (1/8) * w * (w>=wm) * spa ---------
            m_t = []
            for jt in range(NJT):
                spa_s = sp.tile([P, IW], F32, tag="spa_s", bufs=2)
                nc.sync.dma_start(out=spa_s[:],
                                  in_=spa_e[jt * P:(jt + 1) * P, :])
                tt = sp.tile([P, IW], F32, tag="s2")
                nc.vector.scalar_tensor_tensor(tt[:], w_t[jt][:], wm_col[:],
                                               spa_s[:],
                                               op0=ALU.is_ge, op1=ALU.mult)
                m = w_t[jt]
                nc.vector.scalar_tensor_tensor(m[:], w_t[jt][:], DH ** -0.5,
                                               tt[:], op0=ALU.mult,
                                               op1=ALU.mult)
                m_t.append(m)

            # ---------------- attention (head pairs) ----------------
            out_raw = []
            s_rows = rp.tile([H, IW], F32, tag="s_rows")
            for pr in range(4):
                kc, qc = keT[pr], qeT[pr]
                eT = []
                for jt in range(NJT):
                    ps = psA.tile([P, 2 * IW], F32, tag="mm")
                    for hl in range(2):
                        nc.tensor.matmul(
                            ps[:, hl * IW:(hl + 1) * IW],
                            kc[hl * 64:(hl + 1) * 64, jt * P:(jt + 1) * P],
                            qc[hl * 64:(hl + 1) * 64, :],
                            start=True, stop=True)
                    msk = sp.tile([P, 2 * IW], F32, tag="s4")
                    for hl in range(2):
                        nc.vector.tensor_mul(msk[:, hl * IW:(hl + 1) * IW],
                                             ps[:, hl * IW:(hl + 1) * IW],
                                             m_t[jt][:])
                    e = bp.tile([P, 2 * IW], E_DT, tag=f"eT{jt}")
                    nc.scalar.activation(e[:], msk[:], AF.Exp)
                    eT.append(e)
                for hl in range(2):
                    h = 2 * pr + hl
                    po = psB.tile([65, IW], F32, tag="out")
                    for jt in range(NJT):
                        nc.tensor.matmul(
                            po[:],
                            v_aug[jt][:].rearrange("p (hh c) -> p hh c",
                                                   c=65)[:, h, :],
                            eT[jt][:, hl * IW:(hl + 1) * IW],
                            start=(jt == 0), stop=(jt == NJT - 1))
                    s_tmp = sp.tile([P, IW], F32, tag="s2")
                    nc.scalar.copy(s_tmp[64:65, :], po[64:65, :])
                    nc.sync.dma_start(out=s_rows[h:h + 1, :],
                                      in_=s_tmp[64:65, :])
                    orw = bp.tile([64, IW], F32, tag=f"oraw{h}")
                    nc.scalar.copy(orw[:], po[0:64, :])
                    out_raw.append(orw)

            # ---------------- 1/s, normalize, LN stats ----------------
            rec = rp.tile([H, IW], F32, tag="recip_s")
            nc.scalar.activation(rec[:], s_rows[:], AF.Ln)
            nc.scalar.activation(rec[:], rec[:], AF.Exp, scale=-1.0)

            out_true = []
            mean_ps = psR.tile([1, IW], F32, tag="row")
            sq_ps = psR.tile([1, IW], F32, tag="row")
            for h in range(H):
                rb0 = rp.tile([1, IW], F32, tag="rb0", bufs=2)
                nc.sync.dma_start(out=rb0[:], in_=rec[h:h + 1, :])
                rbp = psB.tile([64, IW], F32, tag="out")
                nc.tensor.matmul(rbp[:], ones_row[:, 0:64], rb0[:],
                                 start=True, stop=True)
                ot = out_raw[h]     # normalize in place
                nc.vector.tensor_mul(ot[:], out_raw[h][:], rbp[0:64, :])
                out_true.append(ot)
                nc.tensor.matmul(mean_ps[:], ones_col[0:64, :], ot[:],
                                 start=(h == 0), stop=(h == H - 1))
                sq2 = sp.tile([64, IW], F32, tag="s2")
                nc.scalar.activation(sq2[:], ot[:], AF.Square)
                nc.tensor.matmul(sq_ps[:], ones_col[0:64, :], sq2[:],
                                 start=(h == 0), stop=(h == H - 1))

            mu_row = rp.tile([1, IW], F32, tag="mu_row")
            nc.scalar.mul(mu_row[:], mean_ps[:], 1.0 / INNER)
            mu2 = rp.tile([1, IW], F32, tag="mu2")
            nc.vector.tensor_mul(mu2[:], mu_row[:], mu_row[:])
            var_row = rp.tile([1, IW], F32, tag="var_row")
            nc.vector.scalar_tensor_tensor(var_row[:], sq_ps[:], 1.0 / INNER,
                                           mu2[:], op0=ALU.mult,
                                           op1=ALU.subtract)
            eps_c = rp.tile([1, 1], F32, tag="eps_c")
            nc.vector.memset(eps_c[:], EPS_LN)
            istd_row = rp.tile([1, IW], F32, tag="istd")
            nc.scalar.activation(istd_row[:], var_row[:], AF.Ln, bias=eps_c[:])
            nc.scalar.activation(istd_row[:], istd_row[:], AF.Exp, scale=-0.5)
            nmu_row = rp.tile([1, IW], F32, tag="nmu")
            nc.vector.scalar_tensor_tensor(nmu_row[:], mu_row[:], -1.0,
                                           istd_row[:], op0=ALU.mult,
                                           op1=ALU.mult)
            a_ps = psB.tile([64, IW], F32, tag="out")
            nc.tensor.matmul(a_ps[:], ones_row[:, 0:64], istd_row[:],
                             start=True, stop=True)
            a_bc = cp.tile([64, IW], F32, tag="a_bc")
            nc.scalar.copy(a_bc[:], a_ps[:])
            b_ps = psB.tile([64, IW], F32, tag="out")
            nc.tensor.matmul(b_ps[:], ones_row[:, 0:64], nmu_row[:],
                             start=True, stop=True)
            b_bc = cp.tile([64, IW], F32, tag="b_bc")
            nc.scalar.copy(b_bc[:], b_ps[:])

            # ---------------- LN apply, gelu, out proj ----------------
            fin_ps = psB.tile([D, IW], F32, tag="out")
            for h in range(H):
                y1 = sp.tile([64, IW], F32, tag="s2")
                nc.vector.tensor_mul(y1[:], out_true[h][:], a_bc[:])
                y2 = sp.tile([64, IW], F32, tag="s2")
                nc.vector.tensor_add(y2[:], y1[:], b_bc[:])
                y3 = sp.tile([64, IW], F32, tag="s2")
                nc.vector.tensor_scalar(y3[:], y2[:], lng[:, h:h + 1],
                                        lnb[:, h:h + 1],
                                        op0=ALU.mult, op1=ALU.add)
                g = sp.tile([64, IW], F32, tag="s2")
                nc.scalar.activation(g[:], y3[:], AF.Gelu)
                nc.tensor.matmul(fin_ps[:],
                                 Wout[:, h * D:(h + 1) * D], g[:],
                                 start=(h == 0), stop=(h == H - 1))
            out_sb = cp.tile([D, IW], F32, tag="out_sb")
            nc.scalar.activation(out_sb[:], fin_ps[:], AF.Identity,
                                 bias=bout[:])
            nc.sync.dma_start(out=out_e[:], in_=out_sb[:])

            dbg_sb = rp.tile([1, 16], F32, tag="dbg")
            nc.vector.memset(dbg_sb[:], 0.0)
            nc.scalar.copy(dbg_sb[:, 0:1], ar1_back[:, 0:1])
            nc.scalar.copy(dbg_sb[:, 1:2], ar2_back[:, 0:1])
            nc.sync.dma_start(out=dbg_e[:], in_=dbg_sb[:])

    nc.compile()
    return nc


def prep_in_maps(inputs):
    x = np.ascontiguousarray(inputs["x"], dtype=np.float32)
    W_qkv = np.ascontiguousarray(inputs["W_qkv"], dtype=np.float32)
    b_qkv = np.asarray(inputs["b_qkv"], dtype=np.float32)
    ln_g = np.asarray(inputs["ln_g"], dtype=np.float32)
    ln_b = np.asarray(inputs["ln_b"], dtype=np.float32)
    W_out = np.asarray(inputs["W_out"], dtype=np.float32)
    b_out = np.asarray(inputs["b_out"], dtype=np.float32)

    spa = _spa_full()
    bqk = np.ascontiguousarray(b_qkv[:1024].reshape(8, P).T)
    bv = np.ascontiguousarray(b_qkv[1024:].reshape(1, INNER))
    # head-major: col h holds the 64 per-channel values of head h
    lng = np.ascontiguousarray(ln_g.reshape(H, 64).T)
    lnb = np.ascontiguousarray(ln_b.reshape(H, 64).T)
    # Wout[p, h*64+d] = W_out[h*64+p, d]
    Wout = np.ascontiguousarray(
        W_out.reshape(H, 64, D).transpose(1, 0, 2).reshape(64, H * D))
    bout = np.ascontiguousarray(b_out.reshape(D, 1))

    in_maps = []
    for c in range(NCORES):
        b, ih = c // 2, c % 2
        xb = x[b]
        in_maps.append({
            "xT": np.ascontiguousarray(xb.T),
            "xTmy": np.ascontiguousarray(xb[ih * IW:(ih + 1) * IW].T),
            "Wqkv": W_qkv, "bqk": bqk, "bv": bv,
            "spa": np.ascontiguousarray(spa[:, ih * IW:(ih + 1) * IW]),
            "lng": lng, "lnb": lnb, "Wout": Wout, "bout": bout,
        })
    return in_maps


LAST_RESULTS = None


def kernel(**inputs):
    global LAST_RESULTS
    if "nc" not in _CACHE:
        _CACHE["nc"] = build_nc()
    nc = _CACHE["nc"]
    in_maps = prep_in_maps(inputs)
    trace = bool(int(os.environ.get("KERNEL_TRACE", "0")))
    res = run_bass_kernel_spmd(nc, in_maps, list(range(NCORES)), trace=trace)
    LAST_RESULTS = res
    out = np.empty((B, N, D), dtype=np.float32)
    for c in range(NCORES):
        b, ih = c // 2, c % 2
        out[b, ih * IW:(ih + 1) * IW, :] = res.results[c]["out"].T
    return out
